# revision 1
# baseline (speedup 1.0000x reference)
"""Multi-head attention (B=2, S=4096, E=768, H=12, D=64) on 8 TRN2 NeuronCores.

Sharding: data parallel over batch (2) x tensor parallel over head groups (4):
core c handles batch c//4, heads 3*(c%4) .. 3*(c%4)+2.

Per-core device kernel (fp16 matmul inputs, fp32 accumulation):
  phase 1: Q^T,K^T [192,4096] and V [4096,192] projections from x^T.
    Heads 0,1 of the group are packed at partitions 0-63/64-127 of shared
    Q^T/K^T tiles; head 2's Q and K share one M=128 stationary (host passes
    the combined weight block).
  phase 2: per (q-block of 512, head) block: 3 k-tiles of scores per exp op
    (3 PSUM banks; ScalarE with the 1/8 scale folded in); even/odd k-tiles
    run at partition bases 0/64 (via swapped Q/K duplicates) so adjacent
    K=64 score matmuls occupy different PE row groups and overlap in HW.
    attn@V carries a ones column so the softmax denominator falls out of the
    same matmul; the accumulator spills to SBUF at block end (frees its PSUM
    bank), and normalize (reciprocal + K=1 broadcast matmul) plus the
    row-parallel output projection ride in the next block's slack.
Host: y[b] = sum of the 4 partial y^T.T per batch + b_proj.
"""
import numpy as np

EMBED = 768
SEQ = 4096
NHEAD_CORE = 3          # heads per core
DHEAD = 64
DSL = NHEAD_CORE * DHEAD  # 192: per-core head-dim slice
QB = 512                # q-block (free dim per PSUM bank)
NQB = SEQ // QB         # 8
NKT = SEQ // 128        # 32 k-tiles
NEC = EMBED // 128      # 6 e-chunks
NFT = EMBED // 128      # 6 f-tiles
SCALE = DHEAD ** -0.5

_CACHED = {}


def _build():
    import concourse.bacc as bacc
    import concourse.tile as tile
    from concourse import mybir

    F32 = mybir.dt.float32
    F16 = mybir.dt.float16
    EXP = mybir.ActivationFunctionType.Exp

    nc = bacc.Bacc("TRN2")
    xT_d = nc.dram_tensor("xT", [EMBED, SEQ], F16, kind="ExternalInput")
    wq_d = nc.dram_tensor("wq", [EMBED, 128], F16, kind="ExternalInput")
    wk_d = nc.dram_tensor("wk", [EMBED, 128], F16, kind="ExternalInput")
    wqk2_d = nc.dram_tensor("wqk2", [EMBED, 128], F16, kind="ExternalInput")
    wv_d = nc.dram_tensor("wv", [EMBED, DSL], F16, kind="ExternalInput")
    wp_d = nc.dram_tensor("wp", [DSL, EMBED], F16, kind="ExternalInput")
    yT_d = nc.dram_tensor("yT", [EMBED, SEQ], F32, kind="ExternalOutput")

    with tile.TileContext(nc) as tc:
        with (
            tc.tile_pool(name="persist", bufs=1) as persist,
            tc.tile_pool(name="slab", bufs=12) as slabp,
            tc.tile_pool(name="es", bufs=12) as esp,
            tc.tile_pool(name="att", bufs=2) as attp,
            tc.tile_pool(name="rsbp", bufs=4) as rsbp,
            tc.tile_pool(name="bcp", bufs=4) as bcp,
            tc.tile_pool(name="stage", bufs=3) as stagep,
            tc.tile_pool(name="spill", bufs=2) as spillp,
            tc.tile_pool(name="psA", bufs=2, space="PSUM") as psA,
            tc.tile_pool(name="psB", bufs=1, space="PSUM") as psB,
            tc.tile_pool(name="psC", bufs=1, space="PSUM") as psC,
        ):
            # ---- persistent SBUF ----
            wq_sb = persist.tile([128, NEC, 128], F16, name="wq_sb")
            wk_sb = persist.tile([128, NEC, 128], F16, name="wk_sb")
            wqk2_sb = persist.tile([128, NEC, 128], F16, name="wqk2_sb")
            wv_sb = persist.tile([128, NEC, DSL], F16, name="wv_sb")
            wp_a = persist.tile([128, EMBED], F16, name="wp_a")
            wp_b = persist.tile([128, EMBED], F16, name="wp_b")
            # Q^T/K^T: heads 0,1 at partition halves; head 2 at base 0
            qt01 = persist.tile([128, SEQ], F16, name="qt01")
            kt01 = persist.tile([128, SEQ], F16, name="kt01")
            qt2 = persist.tile([128, SEQ], F16, name="qt2")
            kt2 = persist.tile([128, SEQ], F16, name="kt2")
            # swapped duplicates: [h1 | h0] so every head has Q/K at both
            # partition halves (even k-tiles run at base 0, odd at base 64 ->
            # adjacent matmuls occupy different PE row groups and overlap)
            qtdup = persist.tile([128, SEQ], F16, name="qtdup")
            ktdup = persist.tile([128, SEQ], F16, name="ktdup")
            # V natural layout + ones column: [p, head, kchunk, 66]
            v_sb = persist.tile([128, NHEAD_CORE, NKT, 66], F16, name="v_sb")
            ones_sb = persist.tile([128, 64], F16, name="ones_sb")

            nc.sync.dma_start(out=wq_sb[:], in_=wq_d.rearrange("(c p) d -> p c d", p=128))
            nc.sync.dma_start(out=wk_sb[:], in_=wk_d.rearrange("(c p) d -> p c d", p=128))
            nc.vector.memset(ones_sb[:], 1.0)
            nc.vector.memset(v_sb[:, :, :, 64:66], 1.0)
            # dummy activation: loads the exp table set while ACT is idle
            warm = persist.tile([128, 1], F16, name="warm_sb")
            nc.vector.memset(warm[:], 0.0)
            nc.scalar.activation(out=warm[:], in_=warm[:], func=EXP, scale=1.0)

            # ---- phase 1, pass A: Q^T/K^T for heads 0,1 (gets ACT going fast)
            for sb in range(NQB):  # 8 s-blocks of 512
                qk_ps = psA.tile([128, 1024], F32, name="qkps", tag="psA")
                for e in range(NEC):
                    slab = slabp.tile([128, QB], F16, name="slab", tag="slab")
                    nc.sync.dma_start(
                        out=slab[:],
                        in_=xT_d[128 * e:128 * (e + 1), QB * sb:QB * (sb + 1)],
                    )
                    st = (e == 0)
                    sp = (e == NEC - 1)
                    nc.tensor.matmul(qk_ps[:, 0:QB], wq_sb[:, e, :],
                                     slab[:], start=st, stop=sp)
                    nc.tensor.matmul(qk_ps[:, QB:2 * QB], wk_sb[:, e, :],
                                     slab[:], start=st, stop=sp)
                cols = slice(QB * sb, QB * (sb + 1))
                nc.vector.tensor_copy(qt01[:, cols], qk_ps[:, 0:QB])
                nc.vector.tensor_copy(kt01[:, cols], qk_ps[:, QB:2 * QB])
                nc.vector.tensor_copy(qtdup[64:128, cols], qk_ps[0:64, 0:QB])
                nc.vector.tensor_copy(ktdup[64:128, cols], qk_ps[0:64, QB:2 * QB])

            nc.sync.dma_start(out=wqk2_sb[:], in_=wqk2_d.rearrange("(c p) d -> p c d", p=128))
            nc.sync.dma_start(out=wv_sb[:], in_=wv_d.rearrange("(c p) d -> p c d", p=128))
            nc.sync.dma_start(out=wp_a[:], in_=wp_d[0:128, :])
            nc.sync.dma_start(out=wp_b[0:64, :], in_=wp_d[128:192, :])

            # phase 1, pass B (emitted in bursts inside q-block 0's loop):
            # {Q2|K2} projection + V projection for one s-block
            def pass_b_burst(sb):
                cols = slice(QB * sb, QB * (sb + 1))
                slabs = []
                for e in range(NEC):
                    slab = slabp.tile([128, QB], F16, name="slabB", tag="slab")
                    nc.sync.dma_start(
                        out=slab[:],
                        in_=xT_d[128 * e:128 * (e + 1), cols],
                    )
                    slabs.append(slab)
                qk2_ps = psA.tile([128, 1536], F32, name="qk2ps", tag="psA")
                for e in range(NEC):
                    nc.tensor.matmul(qk2_ps[:, 0:QB], wqk2_sb[:, e, :],
                                     slabs[e][:], start=(e == 0),
                                     stop=(e == NEC - 1))
                nc.vector.tensor_copy(qt2[0:64, cols], qk2_ps[0:64, 0:QB])
                nc.vector.tensor_copy(kt2[0:64, cols], qk2_ps[64:128, 0:QB])
                nc.vector.tensor_copy(qt2[64:128, cols], qk2_ps[0:64, 0:QB])
                nc.vector.tensor_copy(kt2[64:128, cols], qk2_ps[64:128, 0:QB])
                for c in range(4):  # V s-chunks, one PSUM bank at a time
                    v_ps = psC.tile([128, QB], F32, name="vps", tag="psC")
                    for e in range(NEC):
                        nc.tensor.matmul(
                            v_ps[:, 0:DSL],
                            slabs[e][:, 128 * c:128 * (c + 1)],
                            wv_sb[:, e, :],
                            start=(e == 0), stop=(e == NEC - 1))
                    nc.vector.tensor_copy(
                        v_sb[:, :, 4 * sb + c, 0:64],
                        v_ps[:, 0:DSL].rearrange("p (h d) -> p h d", h=NHEAD_CORE),
                    )

            # ---- phase 2: attention + projection ----
            # per-(qb, head) blocks; 3 k-tiles per exp op (3 PSUM banks);
            # attn accumulator spilled to SBUF at block end so psB needs one
            # bank; normalize + projection ride in the next block's slack.
            def normalize_sb(spill, dst):
                """dst = spill[0:64] / spill[64] via recip + K=1 bcast mm."""
                rsb = rsbp.tile([128, QB], F16, name="rsb", tag="rsb")
                with nc.allow_low_precision(reason="fp16 recip feeds bcast mm"):
                    nc.vector.reciprocal(out=rsb[64:65, :], in_=spill[64:65, :])
                ps_bc = psC.tile([128, QB], F32, name="ps_bc", tag="psC")
                nc.tensor.matmul(ps_bc[0:64, :], ones_sb[64:65, 0:64],
                                 rsb[64:65, :], start=True, stop=True)
                bc_sb = bcp.tile([128, QB], F32, name="bc_sb", tag="bc")
                nc.vector.tensor_copy(bc_sb[0:64, :], ps_bc[0:64, :])
                nc.vector.tensor_mul(dst, spill[0:64, :], bc_sb[0:64, :])

            def emit_proj(qb, attA, attB, f):
                qcols = slice(QB * qb, QB * (qb + 1))
                ps_o = psC.tile([128, QB], F32, name="ps_o", tag="psC")
                nc.tensor.matmul(ps_o[:], wp_a[:, 128 * f:128 * (f + 1)],
                                 attA[:], start=True, stop=False)
                nc.tensor.matmul(ps_o[:], wp_b[0:64, 128 * f:128 * (f + 1)],
                                 attB[0:64, :], start=False, stop=True)
                stg = stagep.tile([128, QB], F32, name="stg", tag="stg")
                nc.vector.tensor_copy(stg[:], ps_o[:])
                nc.sync.dma_start(
                    out=yT_d[128 * f:128 * (f + 1), qcols], in_=stg[:])

            # per-head (K even-base-0, Q even, K odd-base-64, Q odd) sources
            def head_srcs(h, kt):
                if kt % 2 == 0:
                    b0 = 0
                    kt_t, qt_t = [(kt01, qt01), (ktdup, qtdup), (kt2, qt2)][h]
                else:
                    b0 = 64
                    kt_t, qt_t = [(ktdup, qtdup), (kt01, qt01), (kt2, qt2)][h]
                return kt_t, qt_t, b0

            GROUPS = [[0, 1]] + [list(range(i, i + 3)) for i in range(2, NKT, 3)]
            att_tiles = {}
            pend_norm = None   # (spill_tile, dst_ap)
            pend_proj = None   # (qb, attA, attB)
            next_burst = 0

            for qb in range(NQB):
                qcols = slice(QB * qb, QB * (qb + 1))
                attA = attp.tile([128, QB], F16, name="attA", tag="attA")
                attB = attp.tile([128, QB], F16, name="attB", tag="attB")
                att_tiles[qb] = (attA, attB)
                for h in range(NHEAD_CORE):
                    ps_att = psB.tile([128, QB], F32, name="ps_att", tag="psB")
                    for gi, group in enumerate(GROUPS):
                        gw = QB * len(group)
                        ps_s = psA.tile([128, 1536], F32, name="ps_s", tag="psA")
                        for i, kt in enumerate(group):
                            kt_t, qt_t, b0 = head_srcs(h, kt)
                            kk = slice(128 * kt, 128 * (kt + 1))
                            nc.tensor.matmul(
                                ps_s[:, QB * i:QB * (i + 1)],
                                kt_t[b0:b0 + 64, kk], qt_t[b0:b0 + 64, qcols],
                                start=True, stop=True)
                        es = esp.tile([128, 1536], F16, name="es", tag="es")
                        nc.scalar.activation(out=es[:, 0:gw], in_=ps_s[:, 0:gw],
                                             func=EXP, scale=SCALE)
                        if qb == 0 and h == 0 and next_burst < NQB:
                            # burst sb covers V k-chunks up to 4*sb+3 >=
                            # group[-1]=3*sb+2, always ahead of the attnV
                            pass_b_burst(next_burst)
                            next_burst += 1
                        for i, kt in enumerate(group):
                            nc.tensor.matmul(
                                ps_att[0:65, :], v_sb[:, h, kt, 0:65],
                                es[:, QB * i:QB * (i + 1)],
                                start=(kt == 0), stop=(kt == NKT - 1),
                                skip_group_check=True)
                        if qb == 0 and h == 0 and 2 <= gi < 6:
                            # h1-side Q/K duplicates, first needed next block
                            srcs = [(qtdup, qt01), (ktdup, kt01)]
                            dt_, st_ = srcs[(gi - 2) % 2]
                            half = slice(0, SEQ // 2) if gi < 4 else slice(SEQ // 2, SEQ)
                            nc.vector.tensor_copy(dt_[0:64, half],
                                                  st_[64:128, half])
                        if gi == 1 and pend_norm is not None:
                            normalize_sb(*pend_norm)
                            pend_norm = None
                        if pend_proj is not None and 3 <= gi < 3 + NFT:
                            emit_proj(pend_proj[0], pend_proj[1], pend_proj[2],
                                      gi - 3)
                            if gi == 3 + NFT - 1:
                                pend_proj = None
                    # spill accumulator to SBUF; frees the psB bank quickly
                    spill = spillp.tile([128, QB], F32, name="spill", tag="spill")
                    nc.vector.tensor_copy(spill[0:65, :], ps_att[0:65, :])
                    if h == 0:
                        dst = attA[0:64, :]
                    elif h == 1:
                        dst = attA[64:128, :]
                    else:
                        dst = attB[0:64, :]
                    pend_norm = (spill, dst)
                    if h == 2:
                        pend_proj = (qb, attA, attB)

            normalize_sb(*pend_norm)
            for f in range(NFT):
                emit_proj(pend_proj[0], pend_proj[1], pend_proj[2], f)

    nc.compile()
    return nc


def _get_nc():
    if "nc" not in _CACHED:
        _CACHED["nc"] = _build()
    return _CACHED["nc"]


def _make_in_maps(x, W_qkv, W_proj):
    f16 = np.float16
    in_maps = []
    for c in range(8):
        b = c // 4
        g = c % 4
        sl = slice(DSL * g, DSL * (g + 1))
        xT = np.ascontiguousarray(x[b].T).astype(f16)
        wqT = np.ascontiguousarray(W_qkv[0:EMBED][sl, :].T)         # [768,192]
        wkT = np.ascontiguousarray(W_qkv[EMBED:2 * EMBED][sl, :].T)
        wvT = np.ascontiguousarray(W_qkv[2 * EMBED:3 * EMBED][sl, :].T)
        wp = np.ascontiguousarray(W_proj[:, sl].T)                  # [192,768]
        wqk2 = np.concatenate([wqT[:, 128:192], wkT[:, 128:192]], axis=1)
        in_maps.append({
            "xT": xT,
            "wq": wqT[:, 0:128].astype(f16),
            "wk": wkT[:, 0:128].astype(f16),
            "wqk2": np.ascontiguousarray(wqk2).astype(f16),
            "wv": wvT.astype(f16),
            "wp": wp.astype(f16),
        })
    return in_maps


def kernel(x, W_qkv, W_proj, b_proj):
    from concourse.bass_utils import run_bass_kernel_spmd

    x = np.asarray(x, dtype=np.float32)
    W_qkv = np.asarray(W_qkv, dtype=np.float32)
    W_proj = np.asarray(W_proj, dtype=np.float32)
    b_proj = np.asarray(b_proj, dtype=np.float32)

    nc = _get_nc()
    in_maps = _make_in_maps(x, W_qkv, W_proj)
    res = run_bass_kernel_spmd(nc, in_maps, core_ids=list(range(8)))

    y = np.zeros((2, SEQ, EMBED), dtype=np.float32)
    for c in range(8):
        y[c // 4] += res.results[c]["yT"].T
    y += b_proj
    return y



# revision 26
# speedup vs baseline: 1.4613x; 1.4613x over previous
"""Multi-head attention (B=2, S=4096, E=768, H=12, D=64) on 8 TRN2 NeuronCores.

Sharding: data parallel over batch (2) x tensor parallel over head groups (4):
core c handles batch c//4, heads 3*(c%4) .. 3*(c%4)+2.

Per-core kernel (fp16 matmul inputs, fp32 accumulation), structured around the
TimelineSim cost model (matmul cost = out-free-size cycles; ACT exp = 0.83ns/el;
engines are independent in-order devices):

  phase 1 (single pass over x^T): per s-block of 512, compute Q^T/K^T for heads
    0,1 (qt01/kt01 [128,4096]), {Q2|K2} packed ([128,4096]), and V in natural
    [s, d] layout with a ones column for the softmax denominator.
  phase 2: flat software pipeline over 24 blocks (qb, h) x 16 groups of 2
    k-tiles:
      PE:   scores (K=64, out [128kt, 512q]) ... attn@V with es as STATIONARY
            and V as moving (out [128q, 65], q-chunks on partitions) -- half
            the matmul cost of the [d, q] formulation; col 64 accumulates the
            softmax denominator via the V ones column.
      exp:  11/16 groups on ACT (true exp); 5/16 on the otherwise-idle GPSIMD
            engine via a tuned fp16 Schraudolph bit trick (one tensor_scalar:
            i16 = trunc(x*A + B), bits reinterpreted as fp16 ~= e^(x/8)).
      DVE:  per-block normalize in [q, d] layout (reciprocal + broadcast mul),
            PSUM->SBUF copies.
      PE:   4 transposes/block bring normalized att back to [d, q] for the
            row-parallel output projection; proj rides in the next qb's slack.
    PSUM: one 3-buf x 2-bank rotating pool (scores/transposes/proj/phase 1)
    + a 2-buf x 1-bank pool (attn accumulators / V).
Host: y[b] = sum of the 4 partial y^T.T per batch + b_proj.
"""
import numpy as np

EMBED = 768
SEQ = 4096
NHEAD_CORE = 3          # heads per core
DHEAD = 64
DSL = NHEAD_CORE * DHEAD  # 192: per-core head-dim slice
QB = 512                # q-block (free dim per PSUM bank)
NQB = SEQ // QB         # 8
NKT = SEQ // 128        # 32 k-tiles
NEC = EMBED // 128      # 6 e-chunks
NG = 16                 # k-tile pair groups per (qb, h) block
SCALE = DHEAD ** -0.5

# fp16 Schraudolph exp. Q/K are pre-scaled by sqrt(A_TR) so scores arrive in
# PSUM already multiplied by A_TR; the DVE trick is then one tensor_scalar:
# i16 = trunc(max(scores + B_TR, 0)) and the bits, read as fp16, approximate
# exp(raw/8). The max-clamp keeps very negative scores from wrapping into the
# fp16 sign bit (underflow degrades gracefully to ~0 via subnormals).
A_TR = SCALE * np.log2(np.e) * 1024.0       # 184.66496...
B_TR = 15301.05                              # tuned for trunc + min rms rel err
QSCALE = float(np.sqrt(A_TR))               # folded into both Q^T and K^T
EXP_SCALE = float(SCALE / A_TR)             # ACT exp scale on A_TR-scaled scores
# groups (of 16 per block) whose exp runs as the bit trick on GPSIMD
DVE_G = (0, 3, 5, 8, 11, 13)

_CACHED = {}


def _build(trick_g=(), dve_g=DVE_G, lag=3, warm_n=16, warm_f=448,
           slab_bufs=4, slab_split=3, es_bufs=6, trick_split=1,
           trick_lag=4, p1_copy_act=False, att_copy_act=False,
           norm_div=False, acc_bufs=1, xtra_in_scores=False,
           v_in_scores=False, norm_mult_pool=False, spill_act=False,
           p1_interleave=False):
    import concourse.bacc as bacc
    import concourse.tile as tile
    from concourse import mybir

    F32 = mybir.dt.float32
    F16 = mybir.dt.float16
    I16 = mybir.dt.int16
    EXP = mybir.ActivationFunctionType.Exp
    MULT = mybir.AluOpType.mult
    ADD = mybir.AluOpType.add

    nc = bacc.Bacc("TRN2")
    xT_d = nc.dram_tensor("xT", [EMBED, SEQ], F16, kind="ExternalInput")
    # weights pre-packed on host: [p, e_chunk * free] so one 128-desc DMA each
    wq_d = nc.dram_tensor("wq", [128, EMBED], F16, kind="ExternalInput")
    wk_d = nc.dram_tensor("wk", [128, EMBED], F16, kind="ExternalInput")
    wqk2_d = nc.dram_tensor("wqk2", [128, EMBED], F16, kind="ExternalInput")
    wv_d = nc.dram_tensor("wv", [128, NEC * DSL], F16, kind="ExternalInput")
    wpa_d = nc.dram_tensor("wpa", [128, EMBED], F16, kind="ExternalInput")
    wpb_d = nc.dram_tensor("wpb", [64, EMBED], F16, kind="ExternalInput")
    id_d = nc.dram_tensor("ident", [128, 128], F16, kind="ExternalInput")
    yT_d = nc.dram_tensor("yT", [EMBED, SEQ], F16, kind="ExternalOutput")

    with tile.TileContext(nc) as tc:
        with (
            tc.tile_pool(name="persist", bufs=1) as persist,
            tc.tile_pool(name="slab", bufs=slab_bufs) as slabp,
            tc.tile_pool(name="es", bufs=es_bufs) as esp,
            tc.tile_pool(name="attsb", bufs=2) as attsp,
            tc.tile_pool(name="att", bufs=2) as attp,
            tc.tile_pool(name="rsb", bufs=2) as rsbp,
            tc.tile_pool(name="stage", bufs=3) as stagep,
            tc.tile_pool(name="spill", bufs=2) as spillp,
            tc.tile_pool(name="ps", bufs=1, space="PSUM") as psp,
        ):
            def ps_scores():
                return psp.tile([128, 1024], F32, name="ps_s", tag="scores",
                                bufs=3)

            def ps_acc(shape):
                return psp.tile(shape, F32, name="ps_acc", tag="acc",
                                bufs=acc_bufs)

            def ps_extra(dtype):
                if xtra_in_scores:
                    return psp.tile([128, 512], dtype, name="ps_x",
                                    tag="scores", bufs=3)
                return psp.tile([128, 512], dtype, name="ps_x", tag="extra",
                                bufs=1)

            # ---- persistent SBUF ----
            wq_sb = persist.tile([128, NEC, 128], F16, name="wq_sb")
            wk_sb = persist.tile([128, NEC, 128], F16, name="wk_sb")
            wqk2_sb = persist.tile([128, NEC, 128], F16, name="wqk2_sb")
            wv_sb = persist.tile([128, NEC, DSL], F16, name="wv_sb")
            wp_a = persist.tile([128, EMBED], F16, name="wp_a")
            wp_b = persist.tile([128, EMBED], F16, name="wp_b")
            qt01 = persist.tile([128, SEQ], F16, name="qt01")
            kt01 = persist.tile([128, SEQ], F16, name="kt01")
            qt2 = persist.tile([64, SEQ], F16, name="qt2")
            kt2 = persist.tile([64, SEQ], F16, name="kt2")
            v_sb = persist.tile([128, NHEAD_CORE, NKT, 66], F16, name="v_sb")
            ident = persist.tile([128, 128], F16, name="ident")
            junk = persist.tile([128, 512], F16, name="junk")

            nc.sync.dma_start(out=ident[:], in_=id_d[:, :])
            nc.sync.dma_start(out=wq_sb[:], in_=wq_d[:, :])
            nc.sync.dma_start(out=wk_sb[:], in_=wk_d[:, :])
            nc.vector.memset(junk[:], 0.0)
            nc.vector.memset(v_sb[:, :, :, 64:66], 1.0)
            # dummy activation: loads the exp table set while ACT is idle
            warm = persist.tile([128, 1], F16, name="warm_sb")
            nc.vector.memset(warm[:], 0.0)
            nc.scalar.activation(out=warm[:], in_=warm[:], func=EXP, scale=1.0)
            # PE p-state warmers: keep PE busy while the first slabs DMA in
            for i in range(warm_n):
                jp = ps_scores()
                nc.tensor.matmul(jp[:, 0:warm_f], junk[:, 0:128],
                                 junk[:, 0:warm_f], start=True, stop=True)

            nc.sync.dma_start(out=wqk2_sb[:], in_=wqk2_d[:, :])
            nc.sync.dma_start(out=wv_sb[:], in_=wv_d[:, :])
            nc.sync.dma_start(out=wp_a[:], in_=wpa_d[:, :])
            nc.sync.dma_start(out=wp_b[0:64, :], in_=wpb_d[:, :])

            # ---- phase 1 pieces (interleavable into early phase 2) ----
            slab_tiles = {}

            def p1_dma(sb):
                cols = slice(QB * sb, QB * (sb + 1))
                slab = slabp.tile([128, NEC, QB], F16, name="slab",
                                  tag="slab")
                ec = NEC // slab_split
                for j in range(slab_split):
                    nc.sync.dma_start(
                        out=slab[:, ec * j:ec * (j + 1), :],
                        in_=xT_d[128 * ec * j:128 * ec * (j + 1), cols]
                        .rearrange("(c p) s -> p c s", p=128))
                slab_tiles[sb] = slab

            def p1_qk(sb):
                cols = slice(QB * sb, QB * (sb + 1))
                slabs = [slab_tiles[sb][:, e, :] for e in range(NEC)]
                qk_ps = ps_scores()
                for e in range(NEC):
                    st, sp = (e == 0), (e == NEC - 1)
                    nc.tensor.matmul(qk_ps[:, 0:QB], wq_sb[:, e, :],
                                     slabs[e][:], start=st, stop=sp)
                    nc.tensor.matmul(qk_ps[:, QB:1024], wk_sb[:, e, :],
                                     slabs[e][:], start=st, stop=sp)
                if p1_copy_act:
                    nc.scalar.mul(qt01[:, cols], qk_ps[:, 0:QB], QSCALE)
                    nc.scalar.mul(kt01[:, cols], qk_ps[:, QB:1024], QSCALE)
                else:
                    nc.vector.tensor_scalar_mul(qt01[:, cols], qk_ps[:, 0:QB],
                                                QSCALE)
                    nc.vector.tensor_scalar_mul(kt01[:, cols],
                                                qk_ps[:, QB:1024], QSCALE)

            def p1_qk2(sb):
                cols = slice(QB * sb, QB * (sb + 1))
                slabs = [slab_tiles[sb][:, e, :] for e in range(NEC)]
                qk2_ps = ps_extra(F32)
                for e in range(NEC):
                    nc.tensor.matmul(qk2_ps[:], wqk2_sb[:, e, :], slabs[e][:],
                                     start=(e == 0), stop=(e == NEC - 1))
                if p1_copy_act:
                    nc.scalar.mul(qt2[:, cols], qk2_ps[0:64, :], QSCALE)
                    nc.scalar.mul(kt2[:, cols], qk2_ps[64:128, :], QSCALE)
                else:
                    nc.vector.tensor_scalar_mul(qt2[:, cols], qk2_ps[0:64, :],
                                                QSCALE)
                    nc.vector.tensor_scalar_mul(kt2[:, cols],
                                                qk2_ps[64:128, :], QSCALE)

            def p1_v(sb, half):
                ceng = nc.scalar.copy if p1_copy_act else nc.vector.tensor_copy
                slabs = [slab_tiles[sb][:, e, :] for e in range(NEC)]
                if v_in_scores:
                    v_ps = psp.tile([128, 2, DSL], F32, name="v_ps",
                                    tag="scores", bufs=3)
                else:
                    v_ps = ps_acc([128, 2, DSL])
                for ci in range(2):
                    c = 2 * half + ci
                    for e in range(NEC):
                        nc.tensor.matmul(
                            v_ps[:, ci, :],
                            slabs[e][:, 128 * c:128 * (c + 1)],
                            wv_sb[:, e, :],
                            start=(e == 0), stop=(e == NEC - 1))
                kt0 = 4 * sb + 2 * half
                ceng(
                    v_sb[:, :, kt0:kt0 + 2, 0:64],
                    v_ps[:].rearrange("p c (h d) -> p h c d", h=3))

            # per-s-block piece lists; consumed in order by the main loop
            p1q = []
            for sb in range(NQB):
                p1q.append((sb, lambda sb=sb: (p1_dma(sb), p1_qk(sb))))
                p1q.append((sb, lambda sb=sb: p1_qk2(sb)))
                p1q.append((sb, lambda sb=sb: p1_v(sb, 0)))
                p1q.append((sb, lambda sb=sb: p1_v(sb, 1)))
            p1_done = [0]  # number of s-blocks fully emitted

            def p1_pop():
                if p1q:
                    sb, fn = p1q.pop(0)
                    fn()
                    if not p1q or p1q[0][0] != sb:
                        p1_done[0] = sb + 1

            def p1_require(sb_req):
                while p1_done[0] <= sb_req and p1q:
                    p1_pop()

            if not p1_interleave:
                while p1q:
                    p1_pop()

            # ---- phase 2: flat pipeline over 24 blocks x 16 groups ----
            NBLK = NQB * NHEAD_CORE
            T = NBLK * NG
            ps_s_tiles = {}
            es_tiles = {}
            ps_att_tiles = {}
            att_sb_tiles = {}
            psT_tiles = {}
            attA_tiles = {}
            attB_tiles = {}

            def emit_scores(t):
                b, g = divmod(t, NG)
                qb, h = divmod(b, NHEAD_CORE)
                qcols = slice(QB * qb, QB * (qb + 1))
                ps_s = ps_scores()
                for i in range(2):
                    kt = 2 * g + i
                    kk = slice(128 * kt, 128 * (kt + 1))
                    if h == 0:
                        st_ap, mv_ap = kt01[0:64, kk], qt01[0:64, qcols]
                    elif h == 1:
                        st_ap, mv_ap = kt01[64:128, kk], qt01[64:128, qcols]
                    else:
                        st_ap, mv_ap = kt2[:, kk], qt2[:, qcols]
                    nc.tensor.matmul(ps_s[:, QB * i:QB * (i + 1)], st_ap,
                                     mv_ap, start=True, stop=True)
                ps_s_tiles[t] = ps_s

            def emit_exp(t):
                g = t % NG
                es = esp.tile([128, 1024], F16, name="es", tag="es")
                ps_s = ps_s_tiles.pop(t)
                if g in trick_g or g in dve_g:
                    n = 1024 // trick_split
                    for j in range(trick_split):
                        nc.vector.tensor_scalar(
                            out=es[:, n * j:n * (j + 1)].bitcast(I16),
                            in0=ps_s[:, n * j:n * (j + 1)],
                            scalar1=float(B_TR), scalar2=0.0,
                            op0=ADD, op1=mybir.AluOpType.max)
                else:
                    nc.scalar.activation(out=es[:], in_=ps_s[:], func=EXP,
                                         scale=EXP_SCALE)
                es_tiles[t] = es

            def emit_attnv(t):
                b, g = divmod(t, NG)
                qb, h = divmod(b, NHEAD_CORE)
                es = es_tiles.pop(t)
                if g == 0:
                    ps_att_tiles[b] = ps_acc([128, 4, 65])
                    # single start covering the whole bank: per-qc starts would
                    # each mark the full zero region and wipe the other groups
                    nc.tensor.matmul(
                        ps_att_tiles[b][:].rearrange("p a b -> p (a b)"),
                        junk[:, 0:128], junk[:, 0:260],
                        start=True, stop=False, skip_group_check=True)
                ps_att = ps_att_tiles[b]
                for i in range(2):
                    kt = 2 * g + i
                    for qc in range(4):
                        nc.tensor.matmul(
                            ps_att[:, qc, 0:65],
                            es[:, QB * i + 128 * qc:QB * i + 128 * (qc + 1)],
                            v_sb[:, h, kt, 0:65],
                            start=False, stop=(kt == NKT - 1),
                            skip_group_check=True)

            def emit_norm(b):
                ps_att = ps_att_tiles.pop(b)
                asb = attsp.tile([128, 4, DHEAD], F16, name="asb", tag="asb")
                if norm_div:
                    dsb = rsbp.tile([128, 4, 1], F32, name="dsb", tag="dsb")
                    nc.vector.tensor_copy(dsb[:], ps_att[:, :, 64:65])
                    rsb = rsbp.tile([128, 4, 1], F32, name="rsb", tag="rsb")
                    nc.vector.reciprocal(out=rsb[:], in_=dsb[:])
                    nc.vector.tensor_tensor(
                        out=asb[:], in0=ps_att[:, :, 0:DHEAD],
                        in1=rsb[:].broadcast_to((128, 4, DHEAD)), op=MULT)
                else:
                    spill = spillp.tile([128, 4, 65], F32, name="spill",
                                        tag="spill")
                    (nc.scalar.copy if spill_act else
                     nc.vector.tensor_copy)(spill[:], ps_att[:])
                    rsb = rsbp.tile([128, 4, 1], F32, name="rsb", tag="rsb")
                    nc.vector.reciprocal(out=rsb[:], in_=spill[:, :, 64:65])
                    meng = nc.gpsimd if norm_mult_pool else nc.vector
                    meng.tensor_tensor(
                        out=asb[:], in0=spill[:, :, 0:DHEAD],
                        in1=rsb[:].broadcast_to((128, 4, DHEAD)), op=MULT)
                att_sb_tiles[b] = asb

            def emit_transposes(b):
                asb = att_sb_tiles.pop(b)
                psT = ps_extra(F16)
                for qc in range(4):
                    nc.tensor.transpose(psT[0:64, 128 * qc:128 * (qc + 1)],
                                        asb[:, qc, :], ident[:])
                psT_tiles[b] = psT

            def emit_copy(b):
                qb, h = divmod(b, NHEAD_CORE)
                psT = psT_tiles.pop(b)
                if h == 0:
                    attA_tiles[qb] = attp.tile([128, QB], F16, name="attA",
                                               tag="attA")
                    dst = attA_tiles[qb][0:64, :]
                elif h == 1:
                    dst = attA_tiles[qb][64:128, :]
                else:
                    attB_tiles[qb] = attp.tile([128, QB], F16, name="attB",
                                               tag="attB")
                    dst = attB_tiles[qb][0:64, :]
                if att_copy_act:
                    nc.scalar.copy(dst, psT[0:64, :])
                else:
                    nc.vector.tensor_copy(dst, psT[0:64, :])

            def emit_proj(qb, f, tail=False):
                qcols = slice(QB * qb, QB * (qb + 1))
                ps_o = ps_extra(F32)
                nc.tensor.matmul(ps_o[:], wp_a[:, 128 * f:128 * (f + 1)],
                                 attA_tiles[qb][:], start=True, stop=False)
                nc.tensor.matmul(ps_o[:], wp_b[0:64, 128 * f:128 * (f + 1)],
                                 attB_tiles[qb][0:64, :], start=False,
                                 stop=True)
                stg = stagep.tile([128, QB], F16, name="stg", tag="stg")
                nc.vector.tensor_copy(stg[:], ps_o[:])
                nc.sync.dma_start(out=yT_d[128 * f:128 * (f + 1), qcols],
                                  in_=stg[:])

            PROJ_G = {8: 0, 11: 1, 14: 2}
            if trick_lag is None:
                trick_lag = lag
            next_av = 0
            def drain_attnv(t):
                nonlocal next_av
                while next_av <= t:
                    u = next_av
                    ul = trick_lag if (u % NG) in trick_g or (u % NG) in dve_g else lag
                    if u + ul > t:
                        break
                    emit_attnv(u)
                    next_av += 1
            for t in range(T):
                b, g = divmod(t, NG)
                qb, h = divmod(b, NHEAD_CORE)
                if qb == 0:
                    p1_require(min(NQB - 1, (2 * g + 1 + 2 * lag) // 4))
                    if b > 0 and g % 2 == 0:
                        p1_pop()
                emit_scores(t)
                emit_exp(t)
                drain_attnv(t)
                if b > 0:
                    if g == 3:
                        emit_norm(b - 1)
                    elif g == 5:
                        emit_transposes(b - 1)
                    elif g == 6:
                        emit_copy(b - 1)
                if qb > 0 and h < 2 and g in PROJ_G:
                    emit_proj(qb - 1, 3 * h + PROJ_G[g])

            while next_av < T:
                emit_attnv(next_av)
                next_av += 1
            emit_norm(NBLK - 1)
            emit_transposes(NBLK - 1)
            emit_copy(NBLK - 1)
            for f in range(6):
                emit_proj(NQB - 1, f, tail=True)

    nc.compile()
    return nc


def _get_nc():
    if "nc" not in _CACHED:
        _CACHED["nc"] = _build()
    return _CACHED["nc"]


def _pack_w(w):
    # [768, F] -> [128, 6*F]: row 128*c + p goes to [p, c*F : (c+1)*F]
    f = w.shape[1]
    return np.ascontiguousarray(
        w.reshape(NEC, 128, f).transpose(1, 0, 2).reshape(128, NEC * f))


def _make_in_maps(x, W_qkv, W_proj):
    f16 = np.float16
    ident = np.eye(128, dtype=f16)
    in_maps = []
    for c in range(8):
        b = c // 4
        g = c % 4
        sl = slice(DSL * g, DSL * (g + 1))
        xT = np.ascontiguousarray(x[b].T).astype(f16)
        wqT = np.ascontiguousarray(W_qkv[0:EMBED][sl, :].T)         # [768,192]
        wkT = np.ascontiguousarray(W_qkv[EMBED:2 * EMBED][sl, :].T)
        wvT = np.ascontiguousarray(W_qkv[2 * EMBED:3 * EMBED][sl, :].T)
        wp = np.ascontiguousarray(W_proj[:, sl].T)                  # [192,768]
        wqk2 = np.concatenate([wqT[:, 128:192], wkT[:, 128:192]], axis=1)
        in_maps.append({
            "xT": xT,
            "wq": _pack_w(wqT[:, 0:128]).astype(f16),
            "wk": _pack_w(wkT[:, 0:128]).astype(f16),
            "wqk2": _pack_w(wqk2).astype(f16),
            "wv": _pack_w(wvT).astype(f16),
            "wpa": np.ascontiguousarray(wp[0:128, :]).astype(f16),
            "wpb": np.ascontiguousarray(wp[128:192, :]).astype(f16),
            "ident": ident,
        })
    return in_maps


def kernel(x, W_qkv, W_proj, b_proj):
    from concourse.bass_utils import run_bass_kernel_spmd

    x = np.asarray(x, dtype=np.float32)
    W_qkv = np.asarray(W_qkv, dtype=np.float32)
    W_proj = np.asarray(W_proj, dtype=np.float32)
    b_proj = np.asarray(b_proj, dtype=np.float32)

    nc = _get_nc()
    in_maps = _make_in_maps(x, W_qkv, W_proj)
    res = run_bass_kernel_spmd(nc, in_maps, core_ids=list(range(8)))

    y = np.zeros((2, SEQ, EMBED), dtype=np.float32)
    for c in range(8):
        y[c // 4] += res.results[c]["yT"].T.astype(np.float32)
    y += b_proj
    return y
